# revision 22
# baseline (speedup 1.0000x reference)
"""2-layer LSTM (B=128, T=256, D=512, H=1024) + linear head + ELU on 8 trn2 cores.

Strategy: time-phased pure data-parallel (zero inter-core communication).
  - Remote-DMA on this platform has ~7us/call latency (size-independent) and
    only tolerates one exact descriptor pattern; any per-step exchange is
    latency-doomed (measured: the TP4 baseline spent ~70% of its time waiting
    on remote h broadcasts). So: no comms at all.
  - Each core owns 16 batch rows and runs the FULL recurrence for them:
    phase 1 = layer-0 for all 256 steps (weights W0+Wh0 in SBUF, 12MB),
    h0_t^T spilled to DRAM; then weights are swapped in-place for
    Wx1+Wh1 (16MB) and phase 2 = layer-1 consumes h0T from DRAM; head at the
    end. All four weight matrices together (28.5MB) would NOT fit SBUF -
    the phase split is what makes DP possible.
  - Per step, gates are computed in a PACKED layout [128, 1024]: partition
    group 32g..32g+16 = gate g (i,f,g,o) for the 16 rows; 4 col-groups of the
    PE run concurrently via tile_position=(0,32g), so the 16-row matmuls still
    stream the full weight bandwidth. h_t is transposed back to [128,16]
    k-tiles on the PE for the next step's stationary operand.
"""

import sys
from contextlib import ExitStack

import ml_dtypes
import numpy as np

for _p in ("/opt/trn_rl_repo", "/root/.axon_site/_ro/trn_rl_repo"):
    if _p not in sys.path:
        sys.path.append(_p)

import concourse.bacc as bacc
import concourse.mybir as mybir
import concourse.tile as tile
from concourse.bass_utils import run_bass_kernel_spmd
from concourse.masks import make_identity

F32 = mybir.dt.float32
BF16 = mybir.dt.bfloat16
AF = mybir.ActivationFunctionType

P = 128
T = 256
D = 512
H = 1024
BR = 256
R = 16          # batch rows per core
NUM_CORES = 8


def _build(nc, n_steps):
    xt_in = nc.dram_tensor("XT", [4, P, T * R], BF16, kind="ExternalInput").ap()
    w1_in = nc.dram_tensor("W1", [12, P, 4 * H], BF16, kind="ExternalInput").ap()
    w2_in = nc.dram_tensor("W2", [16, P, 4 * H], BF16, kind="ExternalInput").ap()
    wbr_in = nc.dram_tensor("Wbr", [8, P, BR], BF16, kind="ExternalInput").ap()
    b0_in = nc.dram_tensor("b0p", [P, H], F32, kind="ExternalInput").ap()
    b1_in = nc.dram_tensor("b1p", [P, H], F32, kind="ExternalInput").ap()
    bbr_in = nc.dram_tensor("bbrp", [R, BR], F32, kind="ExternalInput").ap()
    y_out = nc.dram_tensor("y", [R, BR], F32, kind="ExternalOutput").ap()
    h0t_d = nc.dram_tensor("h0t_d", [T, P, 2 * P], BF16).ap()

    wslot = nc.alloc_sbuf_tensor("wslot", [P, 16, 4 * H], BF16).ap()
    sXT = nc.alloc_sbuf_tensor("sXT", [P, 4, T * R], BF16).ap()
    sb0 = nc.alloc_sbuf_tensor("sb0", [P, H], F32).ap()
    sb1 = nc.alloc_sbuf_tensor("sb1", [P, H], F32).ap()
    sWbr = nc.alloc_sbuf_tensor("sWbr", [P, 8, BR], BF16).ap()
    sbbr = nc.alloc_sbuf_tensor("sbbr", [R, BR], F32).ap()
    ident = nc.alloc_sbuf_tensor("ident", [P, P], F32).ap()
    ring = [nc.alloc_sbuf_tensor(f"ring{j}", [P, 2 * P], BF16).ap() for j in range(2)]
    cst = nc.alloc_sbuf_tensor("cst", [P, 256], F32).ap()
    tg = nc.alloc_sbuf_tensor("tg", [P, 256], F32).ap()
    t1 = nc.alloc_sbuf_tensor("t1", [P, 256], F32).ap()
    t2 = nc.alloc_sbuf_tensor("t2", [P, 256], F32).ap()
    tcn = nc.alloc_sbuf_tensor("tcn", [P, 256], F32).ap()
    hbuf = nc.alloc_sbuf_tensor("hbuf", [P, 256], F32).ap()

    with tile.TileContext(nc) as tc:
        for k in range(4):
            nc.sync.dma_start(out=sXT[:, k], in_=xt_in[k])
        for s in range(12):
            nc.sync.dma_start(out=wslot[:, s], in_=w1_in[s])
        nc.sync.dma_start(out=sb0, in_=b0_in)
        nc.sync.dma_start(out=sb1, in_=b1_in)
        for j in range(8):
            nc.sync.dma_start(out=sWbr[:, j], in_=wbr_in[j])
        nc.sync.dma_start(out=sbbr, in_=bbr_in)
        make_identity(nc, ident)
        nc.vector.memset(ring[1], 0.0)
        nc.vector.memset(cst, 0.0)

        stack = ExitStack()
        psum_pool = stack.enter_context(tc.tile_pool(name="ps", bufs=2, space="PSUM"))
        pt_pool = stack.enter_context(tc.tile_pool(name="pt", bufs=2, space="PSUM"))
        h0_pool = stack.enter_context(tc.tile_pool(name="h0l", bufs=6))

        def stat_ap(buf, l):
            # stationary [128, 16] for logical k-tile l in a ring/h0in buffer
            return buf[:, P * (l % 2) + 32 * (l // 2) : P * (l % 2) + 32 * (l // 2) + R]

        ps_live = {}

        def mm_group(ps, kt_range, stat_fn, slot0, start_group, stop_group):
            n = len(kt_range)
            for i, kt in enumerate(kt_range):
                stat = stat_fn(kt)
                for j in range(4):
                    for c in range(2):
                        nc.tensor.matmul(
                            ps[32 * j : 32 * j + R, 512 * c : 512 * (c + 1)],
                            stat,
                            wslot[:, slot0 + kt,
                                  H * j + 512 * c : H * j + 512 * (c + 1)],
                            start=start_group and (i == 0),
                            stop=stop_group and (i == n - 1),
                            tile_position=(0, 32 * j),
                        )

        def emit_xg(t, n_xg, xg_stat):
            """Start step t's gate accumulation with the xg matmuls.

            PSUM gates [128, 1024]: partition 32j+b = batch row b, hidden
            chunk j (256-wide); column 256g+d = gate g (i,f,o,g' order),
            so every eltwise op runs on all 128 lanes.
            """
            ps = psum_pool.tile([P, H], F32, name="ps")
            ps_live[t] = ps
            mm_group(ps, range(n_xg), xg_stat, 0, True, False)

        def emit_rec(t, rec_slot0):
            ps = ps_live[t]
            ringprev = ring[(t - 1) % 2]
            mm_group(ps, range(8), lambda kt: stat_ap(ringprev, kt),
                     rec_slot0, False, True)

        def emit_eltwise(t, sb):
            # gates += bias; cols: i=[0:256) f=[256:512) o=[512:768) g=[768:)
            ps = ps_live.pop(t)
            nc.vector.tensor_add(ps, ps, sb)
            nc.scalar.activation(ps[:, 0:768], ps[:, 0:768], AF.Sigmoid)
            nc.scalar.activation(tg, ps[:, 768:1024], AF.Tanh)
            nc.vector.tensor_mul(t2, ps[:, 256:512], cst)           # f * c
            nc.vector.tensor_mul(t1, ps[:, 0:256], tg)              # i * g
            nc.vector.tensor_add(cst, t1, t2)
            nc.scalar.activation(tcn, cst, AF.Tanh)
            nc.vector.tensor_mul(hbuf, ps[:, 512:768], tcn)         # o * tanh(c)

        def emit_transposes(t, store_h0t):
            # two full 128x128 transposes: h[:, 128c:+128] -> k-tiles 2j+c
            rt = ring[t % 2]
            for c in range(2):
                pt = pt_pool.tile([P, P], F32, name="pt")
                nc.tensor.transpose(pt[:, :], hbuf[:, P * c : P * (c + 1)],
                                    ident[:, :])
                nc.vector.tensor_copy(rt[:, P * c : P * (c + 1)], pt[:, :])
            if store_h0t:
                nc.sync.dma_start(out=h0t_d[t], in_=rt)

        # ---------------- phase 1: layer 0 ----------------
        def xg_stat_ph1(t):
            return lambda kt, _t=t: sXT[:, kt, R * _t : R * (_t + 1)]

        emit_xg(0, 4, xg_stat_ph1(0))
        for t in range(n_steps):
            emit_rec(t, 4)
            emit_eltwise(t, sb0)
            if t + 1 < n_steps:
                emit_xg(t + 1, 4, xg_stat_ph1(t + 1))
            emit_transposes(t, True)

        # ---------------- weight swap + state reset -------
        for s in range(16):
            nc.sync.dma_start(out=wslot[:, s], in_=w2_in[s])
        nc.vector.memset(ring[1], 0.0)
        nc.vector.memset(cst, 0.0)

        # ---------------- phase 2: layer 1 ----------------
        def xg_stat_ph2(t):
            h0in = h0_pool.tile([P, 2 * P], BF16, name="h0in")
            nc.sync.dma_start(out=h0in, in_=h0t_d[t])
            return lambda kt, _h=h0in: stat_ap(_h, kt)

        emit_xg(0, 8, xg_stat_ph2(0))
        for t in range(n_steps):
            emit_rec(t, 8)
            emit_eltwise(t, sb1)
            if t + 1 < n_steps:
                emit_xg(t + 1, 8, xg_stat_ph2(t + 1))
            emit_transposes(t, False)

        # ---------------- head: ELU(h1_last @ Wbr + bbr) --
        glast = ring[(n_steps - 1) % 2]
        psh = psum_pool.tile([P, H], F32, name="ps")
        for k in range(8):
            nc.tensor.matmul(psh[0:R, 0:BR], stat_ap(glast, k),
                             sWbr[:, k], start=(k == 0), stop=(k == 7),
                             tile_position=(0, 0))
        br = t1[0:R, 0:BR]
        nc.vector.tensor_add(br, psh[0:R, 0:BR], sbbr)
        rl = t2[0:R, 0:BR]
        nc.scalar.activation(rl, br, AF.Relu)
        mn = tg[0:R, 0:BR]
        nc.vector.tensor_sub(mn, br, rl)
        ex = tcn[0:R, 0:BR]
        nc.scalar.activation(ex, mn, AF.Exp)
        s1 = cst[0:R, 0:BR]
        nc.vector.tensor_add(s1, rl, ex)
        yv = hbuf[0:R, 0:BR]
        nc.vector.tensor_scalar_add(yv, s1, -1.0)
        nc.sync.dma_start(out=y_out, in_=yv)
        stack.close()


def build_program(n_steps=T):
    nc = bacc.Bacc("TRN2", target_bir_lowering=False, debug=False,
                   num_devices=NUM_CORES)
    _build(nc, n_steps)
    nc.compile()
    return nc


def _bf(a):
    return np.ascontiguousarray(np.asarray(a, np.float32).astype(ml_dtypes.bfloat16))


def prepare_inputs(X, W_ih0, W_hh0, b_ih0, b_hh0, W_ih1, W_hh1, b_ih1, b_hh1,
                   W_br, b_br, n_steps=T):
    X = np.asarray(X, np.float32)

    def chunk_pack(wt):
        # pytorch cols [i|f|g|o] (1024 each) -> chunk-stacked layout:
        # new col 1024*j + 256*g + d = wt[:, 1024*G(g) + 256*j + d],
        # G = (i,f,o,g). Chunk j lands in PE col-group j; gate g in
        # column range [256g, 256g+256) of the packed psum.
        w4 = wt.reshape(-1, 4, 4, 256)[:, (0, 1, 3, 2)]   # [K, g, j, d]
        return np.transpose(w4, (0, 2, 1, 3)).reshape(-1, 4 * H)

    W0t = chunk_pack(np.asarray(W_ih0, np.float32).T).reshape(4, P, 4 * H)
    Wh0t = chunk_pack(np.asarray(W_hh0, np.float32).T).reshape(8, P, 4 * H)
    Wx1t = chunk_pack(np.asarray(W_ih1, np.float32).T).reshape(8, P, 4 * H)
    Wh1t = chunk_pack(np.asarray(W_hh1, np.float32).T).reshape(8, P, 4 * H)
    Wbrt = np.asarray(W_br, np.float32).T.reshape(8, P, BR)
    w1 = _bf(np.concatenate([W0t, Wh0t], axis=0))
    w2 = _bf(np.concatenate([Wx1t, Wh1t], axis=0))
    wbr = _bf(Wbrt)

    def packed_bias(b):
        arr = np.asarray(b, np.float32).reshape(4, 4, 256)[(0, 1, 3, 2), :, :]
        out = np.zeros((P, H), np.float32)
        for j in range(4):
            for g in range(4):
                out[32 * j : 32 * j + R, 256 * g : 256 * (g + 1)] = arr[g, j][None, :]
        return out

    b0p = packed_bias(np.asarray(b_ih0, np.float32) + np.asarray(b_hh0, np.float32))
    b1p = packed_bias(np.asarray(b_ih1, np.float32) + np.asarray(b_hh1, np.float32))
    bbrp = np.ascontiguousarray(
        np.tile(np.asarray(b_br, np.float32)[None, :], (R, 1)))

    in_maps = []
    for r in range(NUM_CORES):
        Xr = X[R * r : R * (r + 1), :n_steps]          # [16, n, 512]
        XT = Xr.transpose(2, 1, 0).reshape(D, n_steps * R)   # [512, n*16]
        if n_steps < T:
            XT = np.concatenate(
                [XT, np.zeros((D, (T - n_steps) * R), np.float32)], axis=1)
        in_maps.append({
            "XT": _bf(XT.reshape(4, P, T * R)),
            "W1": w1,
            "W2": w2,
            "Wbr": wbr,
            "b0p": np.ascontiguousarray(b0p),
            "b1p": np.ascontiguousarray(b1p),
            "bbrp": bbrp,
        })
    return in_maps


_cached_nc = None


def kernel(**inputs):
    global _cached_nc
    if _cached_nc is None:
        _cached_nc = build_program(T)
    in_maps = prepare_inputs(**inputs, n_steps=T)
    res = run_bass_kernel_spmd(_cached_nc, in_maps, list(range(NUM_CORES)))
    out = np.concatenate([res.results[r]["y"] for r in range(NUM_CORES)], axis=0)
    return out.astype(np.float32)


# revision 26
# speedup vs baseline: 1.4494x; 1.4494x over previous
"""2-layer LSTM (B=128, T=256, D=512, H=1024) + linear head + ELU on 8 trn2 cores.

Strategy: time-phased pure data-parallel (zero inter-core communication).
  - Remote-DMA on this platform has ~7us/call latency (size-independent) and
    only tolerates one exact descriptor pattern; any per-step exchange is
    latency-doomed (measured: the TP4 baseline spent ~70% of its time waiting
    on remote h broadcasts). So: no comms at all.
  - Each core owns 16 batch rows and runs the FULL recurrence for them:
    phase 1 = layer-0 for all 256 steps (weights W0+Wh0 in SBUF, 12MB),
    h0_t^T spilled to DRAM; then weights are swapped in-place for
    Wx1+Wh1 (16MB) and phase 2 = layer-1 consumes h0T from DRAM; head at the
    end. All four weight matrices together (28.5MB) would NOT fit SBUF -
    the phase split is what makes DP possible.
  - Per step, gates are computed in a PACKED layout [128, 1024]: partition
    group 32g..32g+16 = gate g (i,f,g,o) for the 16 rows; 4 col-groups of the
    PE run concurrently via tile_position=(0,32g), so the 16-row matmuls still
    stream the full weight bandwidth. h_t is transposed back to [128,16]
    k-tiles on the PE for the next step's stationary operand.
"""

import sys
from contextlib import ExitStack

import ml_dtypes
import numpy as np

for _p in ("/opt/trn_rl_repo", "/root/.axon_site/_ro/trn_rl_repo"):
    if _p not in sys.path:
        sys.path.append(_p)

import concourse.bacc as bacc
import concourse.mybir as mybir
import concourse.tile as tile
from concourse.bass_utils import run_bass_kernel_spmd
from concourse.masks import make_identity

F32 = mybir.dt.float32
BF16 = mybir.dt.bfloat16
AF = mybir.ActivationFunctionType

P = 128
T = 256
D = 512
H = 1024
BR = 256
R = 16          # batch rows per core
NUM_CORES = 8


def _build(nc, n_steps):
    xt_in = nc.dram_tensor("XT", [4, P, T * R], BF16, kind="ExternalInput").ap()
    w1_in = nc.dram_tensor("W1", [13, P, 4 * H], BF16, kind="ExternalInput").ap()
    w2_in = nc.dram_tensor("W2", [17, P, 4 * H], BF16, kind="ExternalInput").ap()
    wbr_in = nc.dram_tensor("Wbr", [8, P, BR], BF16, kind="ExternalInput").ap()
    bbr_in = nc.dram_tensor("bbrp", [R, BR], F32, kind="ExternalInput").ap()
    y_out = nc.dram_tensor("y", [R, BR], F32, kind="ExternalOutput").ap()
    h0t_d = nc.dram_tensor("h0t_d", [T, P, 2 * P], BF16).ap()

    wslot = nc.alloc_sbuf_tensor("wslot", [P, 17, 4 * H], BF16).ap()
    e0 = nc.alloc_sbuf_tensor("e0", [P, R], BF16).ap()
    sXT = nc.alloc_sbuf_tensor("sXT", [P, 4, T * R], BF16).ap()
    sWbr = nc.alloc_sbuf_tensor("sWbr", [P, 8, BR], BF16).ap()
    sbbr = nc.alloc_sbuf_tensor("sbbr", [R, BR], F32).ap()
    ident = nc.alloc_sbuf_tensor("ident", [P, P], F32).ap()
    ring = [nc.alloc_sbuf_tensor(f"ring{j}", [P, 2 * P], BF16).ap() for j in range(2)]
    cst = nc.alloc_sbuf_tensor("cst", [P, 256], F32).ap()
    tg = nc.alloc_sbuf_tensor("tg", [P, 256], F32).ap()
    t1 = nc.alloc_sbuf_tensor("t1", [P, 256], F32).ap()
    t2 = nc.alloc_sbuf_tensor("t2", [P, 256], F32).ap()
    tcn = nc.alloc_sbuf_tensor("tcn", [P, 256], F32).ap()
    hbuf = nc.alloc_sbuf_tensor("hbuf", [P, 256], F32).ap()

    with tile.TileContext(nc) as tc:
        for k in range(4):
            nc.sync.dma_start(out=sXT[:, k], in_=xt_in[k])
        for s in range(13):
            nc.sync.dma_start(out=wslot[:, s], in_=w1_in[s])
        for j in range(8):
            nc.sync.dma_start(out=sWbr[:, j], in_=wbr_in[j])
        nc.sync.dma_start(out=sbbr, in_=bbr_in)
        make_identity(nc, ident)
        nc.vector.memset(e0, 0.0)
        nc.vector.memset(e0[0:1, :], 1.0)
        nc.vector.memset(ring[1], 0.0)
        nc.vector.memset(cst, 0.0)

        stack = ExitStack()
        psum_pool = stack.enter_context(tc.tile_pool(name="ps", bufs=3, space="PSUM"))
        pt_pool = stack.enter_context(tc.tile_pool(name="pt", bufs=2, space="PSUM"))
        h0_pool = stack.enter_context(tc.tile_pool(name="h0l", bufs=6))

        def stat_ap(buf, l):
            # stationary [128, 16] for logical k-tile l in a ring/h0in buffer
            return buf[:, P * (l % 2) + 32 * (l // 2) : P * (l % 2) + 32 * (l // 2) + R]

        ps_live = {}

        def mm_group(ps, kt_range, stat_fn, slot0, start_group, stop_group):
            n = len(kt_range)
            for i, kt in enumerate(kt_range):
                stat = stat_fn(kt)
                for j in range(4):
                    for c in range(2):
                        nc.tensor.matmul(
                            ps[32 * j : 32 * j + R, 512 * c : 512 * (c + 1)],
                            stat,
                            wslot[:, slot0 + kt,
                                  H * j + 512 * c : H * j + 512 * (c + 1)],
                            start=start_group and (i == 0),
                            stop=stop_group and (i == n - 1),
                            tile_position=(0, 32 * j),
                        )

        def emit_xg(t, n_xg, xg_stat, bias_slot):
            """Start step t's gate accumulation: bias k-tile + xg matmuls.

            PSUM gates [128, 1024]: partition 32j+b = batch row b, hidden
            chunk j (256-wide); column 256g+d = gate g (i,f,o,g' order),
            so every eltwise op runs on all 128 lanes. The bias is a matmul
            with stationary e0 (ones on contraction row 0) against a weight
            row holding the packed bias -- keeps it off the DVE chain.
            """
            ps = psum_pool.tile([P, H], F32, name="ps")
            ps_live[t] = ps
            mm_group(ps, [bias_slot - 0], lambda kt: e0, 0, True, False)
            mm_group(ps, range(n_xg), xg_stat, 0, False, False)

        def emit_rec(t, rec_slot0):
            ps = ps_live[t]
            ringprev = ring[(t - 1) % 2]
            mm_group(ps, range(8), lambda kt: stat_ap(ringprev, kt),
                     rec_slot0, False, True)

        def emit_eltwise(t):
            # cols: i=[0:256) f=[256:512) o=[512:768) g=[768:)
            ps = ps_live.pop(t)
            nc.scalar.activation(ps[:, 0:768], ps[:, 0:768], AF.Sigmoid)
            nc.scalar.activation(tg, ps[:, 768:1024], AF.Tanh)
            nc.vector.tensor_mul(t2, ps[:, 256:512], cst)           # f * c
            nc.vector.tensor_mul(t1, ps[:, 0:256], tg)              # i * g
            nc.vector.tensor_add(cst, t1, t2)
            nc.scalar.activation(tcn, cst, AF.Tanh)
            nc.vector.tensor_mul(hbuf, ps[:, 512:768], tcn)         # o * tanh(c)

        def emit_transposes(t, store_h0t):
            # two full 128x128 transposes: h[:, 128c:+128] -> k-tiles 2j+c
            rt = ring[t % 2]
            for c in range(2):
                pt = pt_pool.tile([P, P], F32, name="pt")
                nc.tensor.transpose(pt[:, :], hbuf[:, P * c : P * (c + 1)],
                                    ident[:, :])
                nc.vector.tensor_copy(rt[:, P * c : P * (c + 1)], pt[:, :])
            if store_h0t:
                nc.sync.dma_start(out=h0t_d[t], in_=rt)

        # ---------------- phase 1: layer 0 ----------------
        def xg_stat_ph1(t):
            return lambda kt, _t=t: sXT[:, kt, R * _t : R * (_t + 1)]

        emit_xg(0, 4, xg_stat_ph1(0), 12)
        if n_steps > 1:
            emit_xg(1, 4, xg_stat_ph1(1), 12)
        for t in range(n_steps):
            emit_rec(t, 4)
            emit_eltwise(t)
            if t + 2 < n_steps:
                emit_xg(t + 2, 4, xg_stat_ph1(t + 2), 12)
            emit_transposes(t, True)

        # ---------------- weight swap + state reset -------
        for s in range(17):
            nc.sync.dma_start(out=wslot[:, s], in_=w2_in[s])
        nc.vector.memset(ring[1], 0.0)
        nc.vector.memset(cst, 0.0)

        # ---------------- phase 2: layer 1 ----------------
        def xg_stat_ph2(t):
            h0in = h0_pool.tile([P, 2 * P], BF16, name="h0in")
            nc.sync.dma_start(out=h0in, in_=h0t_d[t])
            return lambda kt, _h=h0in: stat_ap(_h, kt)

        emit_xg(0, 8, xg_stat_ph2(0), 16)
        if n_steps > 1:
            emit_xg(1, 8, xg_stat_ph2(1), 16)
        for t in range(n_steps):
            emit_rec(t, 8)
            emit_eltwise(t)
            if t + 2 < n_steps:
                emit_xg(t + 2, 8, xg_stat_ph2(t + 2), 16)
            emit_transposes(t, False)

        # ---------------- head: ELU(h1_last @ Wbr + bbr) --
        glast = ring[(n_steps - 1) % 2]
        psh = psum_pool.tile([P, H], F32, name="ps")
        for k in range(8):
            nc.tensor.matmul(psh[0:R, 0:BR], stat_ap(glast, k),
                             sWbr[:, k], start=(k == 0), stop=(k == 7),
                             tile_position=(0, 0))
        br = t1[0:R, 0:BR]
        nc.vector.tensor_add(br, psh[0:R, 0:BR], sbbr)
        rl = t2[0:R, 0:BR]
        nc.scalar.activation(rl, br, AF.Relu)
        mn = tg[0:R, 0:BR]
        nc.vector.tensor_sub(mn, br, rl)
        ex = tcn[0:R, 0:BR]
        nc.scalar.activation(ex, mn, AF.Exp)
        s1 = cst[0:R, 0:BR]
        nc.vector.tensor_add(s1, rl, ex)
        yv = hbuf[0:R, 0:BR]
        nc.vector.tensor_scalar_add(yv, s1, -1.0)
        nc.sync.dma_start(out=y_out, in_=yv)
        stack.close()


def build_program(n_steps=T):
    nc = bacc.Bacc("TRN2", target_bir_lowering=False, debug=False,
                   num_devices=NUM_CORES)
    _build(nc, n_steps)
    nc.compile()
    return nc


def _bf(a):
    return np.ascontiguousarray(np.asarray(a, np.float32).astype(ml_dtypes.bfloat16))


def prepare_inputs(X, W_ih0, W_hh0, b_ih0, b_hh0, W_ih1, W_hh1, b_ih1, b_hh1,
                   W_br, b_br, n_steps=T):
    X = np.asarray(X, np.float32)

    def chunk_pack(wt):
        # pytorch cols [i|f|g|o] (1024 each) -> chunk-stacked layout:
        # new col 1024*j + 256*g + d = wt[:, 1024*G(g) + 256*j + d],
        # G = (i,f,o,g). Chunk j lands in PE col-group j; gate g in
        # column range [256g, 256g+256) of the packed psum.
        w4 = wt.reshape(-1, 4, 4, 256)[:, (0, 1, 3, 2)]   # [K, g, j, d]
        return np.transpose(w4, (0, 2, 1, 3)).reshape(-1, 4 * H)

    W0t = chunk_pack(np.asarray(W_ih0, np.float32).T).reshape(4, P, 4 * H)
    Wh0t = chunk_pack(np.asarray(W_hh0, np.float32).T).reshape(8, P, 4 * H)
    Wx1t = chunk_pack(np.asarray(W_ih1, np.float32).T).reshape(8, P, 4 * H)
    Wh1t = chunk_pack(np.asarray(W_hh1, np.float32).T).reshape(8, P, 4 * H)
    Wbrt = np.asarray(W_br, np.float32).T.reshape(8, P, BR)
    wbr = _bf(Wbrt)

    def bias_wrow(b):
        # weight plane for the bias matmul: contraction row 0 = packed bias
        out = np.zeros((1, P, 4 * H), np.float32)
        out[0, 0, :] = chunk_pack(np.asarray(b, np.float32)[None, :])[0]
        return out

    b0w = bias_wrow(np.asarray(b_ih0, np.float32) + np.asarray(b_hh0, np.float32))
    b1w = bias_wrow(np.asarray(b_ih1, np.float32) + np.asarray(b_hh1, np.float32))
    w1 = _bf(np.concatenate([W0t, Wh0t, b0w], axis=0))
    w2 = _bf(np.concatenate([Wx1t, Wh1t, b1w], axis=0))
    bbrp = np.ascontiguousarray(
        np.tile(np.asarray(b_br, np.float32)[None, :], (R, 1)))

    in_maps = []
    for r in range(NUM_CORES):
        Xr = X[R * r : R * (r + 1), :n_steps]          # [16, n, 512]
        XT = Xr.transpose(2, 1, 0).reshape(D, n_steps * R)   # [512, n*16]
        if n_steps < T:
            XT = np.concatenate(
                [XT, np.zeros((D, (T - n_steps) * R), np.float32)], axis=1)
        in_maps.append({
            "XT": _bf(XT.reshape(4, P, T * R)),
            "W1": w1,
            "W2": w2,
            "Wbr": wbr,
            "bbrp": bbrp,
        })
    return in_maps


_cached_nc = None


def kernel(**inputs):
    global _cached_nc
    if _cached_nc is None:
        _cached_nc = build_program(T)
    in_maps = prepare_inputs(**inputs, n_steps=T)
    res = run_bass_kernel_spmd(_cached_nc, in_maps, list(range(NUM_CORES)))
    out = np.concatenate([res.results[r]["y"] for r in range(NUM_CORES)], axis=0)
    return out.astype(np.float32)
